# revision 17
# baseline (speedup 1.0000x reference)
"""AdaConv2d Trainium2 kernel — 8-core data-parallel (one sample per core).

Per-core pipeline (sample b on core b):
  1. stream x[b] (f32) into SBUF, computing instance-norm stats (bn_stats)
     while casting into a reflect-padded bf16 buffer xp [128p, 130, 130] x2 blocks
  2. normalize xp in place: (x - mean) * rsqrt(var + eps)
  3. adaptive grouped 3x3+1x1 conv as 9 block-diagonal 128x128 matmuls per
     4-row chunk (composite weights = pointwise @ spatial, computed on device)
  4. + per-channel bias -> reflect-padded bf16 z buffer zp
  5. final dense 3x3 conv 256->256 as 18 accumulating matmuls per 4-row chunk
  6. + conv bias -> DMA out (f32)

Host side does layout-only prep (shard, transpose, block-diag scatter of the
weight tensors); all arithmetic runs on device.
"""

import sys

sys.path.insert(0, "/opt/trn_rl_repo")

import numpy as np

import concourse.bass as bass
import concourse.tile as tile
from concourse import mybir
from concourse.bass_utils import run_bass_kernel_spmd

F32 = mybir.dt.float32
BF16 = mybir.dt.bfloat16

B = 8
C = 256
H = W = 128
HW = H * W
NB = 2  # channel blocks of 128
PB = H + 2  # padded extent (reflect pad 1)
NOFF = 9
EPS = 1e-5

_CACHE = {}
LAST_EXEC_NS = None


def _build():
    nc = bass.Bass(trn_type="TRN2", debug=False)

    x_d = nc.declare_dram_parameter("x", [C, HW], F32, False)
    wsbd_d = nc.declare_dram_parameter("wsbd", [NB, 128, NOFF, 128], F32, False)
    wptbd_d = nc.declare_dram_parameter("wptbd", [NB, 128, 128], F32, False)
    cwt_d = nc.declare_dram_parameter("cwt", [NB, 128, NOFF, NB, 128], F32, False)
    biasp_d = nc.declare_dram_parameter("biasp", [NB, 128, 1], F32, False)
    convbp_d = nc.declare_dram_parameter("convbp", [NB, 128, 1], F32, False)
    out_d = nc.declare_dram_parameter("out", [C, HW], F32, True)

    with tile.TileContext(nc) as tc:
        with (
            tc.tile_pool(name="big", bufs=1) as big,
            tc.tile_pool(name="wconst", bufs=1) as wconst,
            tc.tile_pool(name="pad", bufs=3) as padpool,
            tc.tile_pool(name="xstream", bufs=3) as xstream,
            tc.tile_pool(name="ostage", bufs=4) as opool,
            tc.tile_pool(name="psum", bufs=6, space="PSUM") as psum,
        ):
            # ---------- weight prologue ----------
            # final conv weights: [ic, off, ocb, oc] f32 per icb -> bf16
            wbf = []
            for icb in range(NB):
                # own slot per staged weight tile: DMA-landing slots must not
                # be reused (slot-reuse WAW would add a 2nd wait; DMA insts
                # support only one)
                wf32 = wconst.tile([128, NOFF, NB, 128], F32, name=f"wf32_{icb}")
                nc.gpsimd.dma_start(out=wf32, in_=cwt_d[icb])
                wb = wconst.tile([128, NOFF, NB, 128], BF16, name=f"wbf_{icb}")
                nc.vector.tensor_copy(out=wb, in_=wf32)
                wbf.append(wb)

            # composite adaptive weights: lhsta[cb][off][j, o] (bf16, block-diag)
            # NOTE: everything a matmul reads must be produced by DVE (one
            # engine -> one coalesced sem wait; Matmult supports only 1 wait).
            lhsta = [[None] * NOFF for _ in range(NB)]
            for cb in range(NB):
                wsf0 = wconst.tile([128, NOFF, 128], F32, name=f"wsf0_{cb}")
                nc.gpsimd.dma_start(out=wsf0, in_=wsbd_d[cb])
                wpf0 = wconst.tile([128, 128], F32, name=f"wpf0_{cb}")
                nc.gpsimd.dma_start(out=wpf0, in_=wptbd_d[cb])
                wsf = wconst.tile([128, NOFF, 128], F32, name=f"wsf_{cb}")
                nc.vector.tensor_copy(out=wsf, in_=wsf0)
                wpf = wconst.tile([128, 128], F32, name=f"wpf_{cb}")
                nc.vector.tensor_copy(out=wpf, in_=wpf0)
                for off in range(NOFF):
                    ps = psum.tile([128, 128], F32, tag="ps", name=f"cps_{cb}_{off}")
                    nc.tensor.matmul(
                        ps, lhsT=wsf[:, off, :], rhs=wpf, start=True, stop=True
                    )
                    lt = wconst.tile([128, 128], BF16, name=f"lhsta_{cb}_{off}")
                    nc.vector.tensor_copy(out=lt, in_=ps)
                    lhsta[cb][off] = lt

            bias_sb = []
            convb_sb = []
            for cb in range(NB):
                bt = wconst.tile([128, 1], F32, name=f"bias_{cb}")
                nc.gpsimd.dma_start(out=bt, in_=biasp_d[cb])
                bias_sb.append(bt)
                ct = wconst.tile([128, 1], F32, name=f"convb_{cb}")
                nc.gpsimd.dma_start(out=ct, in_=convbp_d[cb])
                convb_sb.append(ct)

            eps_sb = wconst.tile([128, 1], F32, name="eps")
            nc.vector.memset(eps_sb, EPS)

            # ---------- stream x in, stats, cast to padded bf16 ----------
            # xp/zp share 3 slots: xp[cb] is dead once the adaptive conv of
            # block cb finishes, so zp[1] can reuse xp[0]'s slot.
            xp = [
                padpool.tile([128, PB, PB], BF16, tag="pad", name=f"xp_{cb}")
                for cb in range(NB)
            ]
            zp = [
                padpool.tile([128, PB, PB], BF16, tag="pad", name=f"zp_{cb}")
                for cb in range(NB)
            ]

            NCHUNK = 8  # 16 image rows (2048 elems) per chunk
            ROWS = H // NCHUNK
            for cb in range(NB):
                stats = wconst.tile([128, 4 * NCHUNK, 6], F32, name=f"stats_{cb}")
                for ch in range(NCHUNK):
                    xc = xstream.tile([128, ROWS, W], F32, tag="xc", name=f"xc_{cb}_{ch}")
                    nc.gpsimd.dma_start(
                        out=xc,
                        in_=x_d[cb * 128 : (cb + 1) * 128, ch * ROWS * W : (ch + 1) * ROWS * W],
                    )
                    xcf = xc.rearrange("p a b -> p (a b)")
                    for s in range(4):
                        nc.vector.bn_stats(
                            out=stats[:, ch * 4 + s, :],
                            in_=xcf[:, s * 512 : (s + 1) * 512],
                        )
                    # cast into padded interior (DVE: matmuls read xp)
                    nc.vector.tensor_copy(
                        out=xp[cb][:, 1 + ch * ROWS : 1 + (ch + 1) * ROWS, 1 : 1 + W],
                        in_=xc,
                    )

                mv = wconst.tile([128, 2], F32, name=f"mv_{cb}")
                nc.vector.bn_aggr(out=mv, in_=stats)
                rstd = wconst.tile([128, 1], F32, name=f"rstd_{cb}")
                nc.scalar.activation(
                    out=rstd,
                    in_=mv[:, 1:2],
                    func=mybir.ActivationFunctionType.Sqrt,
                    bias=eps_sb,
                )
                nc.vector.reciprocal(out=rstd, in_=rstd)

                # reflect borders (raw values; normalize afterwards hits them too)
                p = xp[cb]
                nc.vector.tensor_copy(out=p[:, 1 : 1 + H, 0:1], in_=p[:, 1 : 1 + H, 2:3])
                nc.vector.tensor_copy(
                    out=p[:, 1 : 1 + H, PB - 1 : PB], in_=p[:, 1 : 1 + H, PB - 3 : PB - 2]
                )
                nc.vector.tensor_copy(out=p[:, 0:1, :], in_=p[:, 2:3, :])
                nc.vector.tensor_copy(out=p[:, PB - 1 : PB, :], in_=p[:, PB - 3 : PB - 2, :])

                # normalize in place: (x - mean) * rstd
                nc.vector.tensor_scalar(
                    out=p[:, :, :],
                    in0=p[:, :, :],
                    scalar1=mv[:, 0:1],
                    scalar2=rstd,
                    op0=mybir.AluOpType.subtract,
                    op1=mybir.AluOpType.mult,
                )

            # ---------- adaptive conv (9 block-diag matmuls / 4-row chunk) ----------
            RC = 4  # rows per chunk -> N = 512
            NRC = H // RC
            for cb in range(NB):
                for ck in range(NRC):
                    r = ck * RC
                    ps = psum.tile([128, RC, W], F32, tag="ps", name=f"aps_{cb}_{ck}")
                    for off in range(NOFF):
                        dy, dx = off // 3 - 1, off % 3 - 1
                        rhs = xp[cb][
                            :, r + 1 + dy : r + 1 + RC + dy, 1 + dx : 1 + W + dx
                        ]
                        nc.tensor.matmul(
                            ps,
                            lhsT=lhsta[cb][off],
                            rhs=rhs,
                            start=(off == 0),
                            stop=(off == NOFF - 1),
                        )
                    # bias add + cast into padded z interior
                    nc.vector.tensor_scalar_add(
                        out=zp[cb][:, r + 1 : r + 1 + RC, 1 : 1 + W],
                        in0=ps,
                        scalar1=bias_sb[cb],
                    )
                    # side border cols for these rows
                    z = zp[cb]
                    nc.vector.tensor_copy(
                        out=z[:, r + 1 : r + 1 + RC, 0:1],
                        in_=z[:, r + 1 : r + 1 + RC, 2:3],
                    )
                    nc.vector.tensor_copy(
                        out=z[:, r + 1 : r + 1 + RC, PB - 1 : PB],
                        in_=z[:, r + 1 : r + 1 + RC, PB - 3 : PB - 2],
                    )
                z = zp[cb]
                nc.vector.tensor_copy(out=z[:, 0:1, :], in_=z[:, 2:3, :])
                nc.vector.tensor_copy(out=z[:, PB - 1 : PB, :], in_=z[:, PB - 3 : PB - 2, :])

            # ---------- final dense 3x3 conv (18 matmuls / 4-row chunk) ----------
            for ocb in range(NB):
                for ck in range(NRC):
                    r = ck * RC
                    ps = psum.tile([128, RC, W], F32, tag="ps", name=f"fps_{ocb}_{ck}")
                    k = 0
                    for icb in range(NB):
                        for off in range(NOFF):
                            dy, dx = off // 3 - 1, off % 3 - 1
                            rhs = zp[icb][
                                :, r + 1 + dy : r + 1 + RC + dy, 1 + dx : 1 + W + dx
                            ]
                            nc.tensor.matmul(
                                ps,
                                lhsT=wbf[icb][:, off, ocb, :],
                                rhs=rhs,
                                start=(k == 0),
                                stop=(k == 2 * NOFF - 1),
                            )
                            k += 1
                    ost = opool.tile([128, RC, W], F32, tag="ost", name=f"ost_{ocb}_{ck}")
                    nc.vector.tensor_scalar_add(out=ost, in0=ps, scalar1=convb_sb[ocb])
                    nc.gpsimd.dma_start(
                        out=out_d[ocb * 128 : (ocb + 1) * 128, r * W : (r + RC) * W],
                        in_=ost,
                    )

    _split_waits(nc)
    return nc


def _split_waits(nc, max_waits=1):
    """Walrus codegen allows only one embedded sync-wait per instruction
    (except SyncE drains). Move excess waits onto injected same-engine NOPs
    placed immediately before the over-constrained instruction."""
    n_new = 0
    for f in nc.m.functions:
        for bb in f.blocks:
            new_insts = []
            changed = False
            for inst in bb.instructions:
                si = inst.sync_info
                if si is not None and si.on_wait and len(si.on_wait) > max_waits:
                    extra = list(si.on_wait)[:-max_waits]
                    keep = list(si.on_wait)[-max_waits:]
                    for w in extra:
                        nop = mybir.InstNoOp(name=f"waitnop-{n_new}", ins=[], outs=[])
                        nop.engine = inst.engine
                        nop.sync_info = mybir.SyncInfo(on_wait=[w], on_update=[])
                        new_insts.append(nop)
                        n_new += 1
                    inst.sync_info = mybir.SyncInfo(
                        on_wait=keep, on_update=list(si.on_update)
                    )
                    changed = True
                new_insts.append(inst)
            if changed:
                bb.instructions = new_insts
    return n_new


def _prep_inputs(x, w_spatial, w_pointwise, bias, conv_w, conv_b):
    """Layout-only host prep: shard + transpose/scatter weights."""
    x = np.asarray(x, np.float32)
    w_spatial = np.asarray(w_spatial, np.float32)
    w_pointwise = np.asarray(w_pointwise, np.float32)
    bias = np.asarray(bias, np.float32)
    conv_w = np.asarray(conv_w, np.float32)
    conv_b = np.asarray(conv_b, np.float32)

    # cwt[icb, ic, off, ocb, oc] = conv_w[ocb*128+oc, icb*128+ic, off]
    cw = conv_w.reshape(C, C, NOFF)
    cwt = np.ascontiguousarray(
        cw.reshape(NB, 128, NB, 128, NOFF).transpose(2, 3, 4, 0, 1), np.float32
    )
    convbp = np.ascontiguousarray(conv_b.reshape(NB, 128, 1), np.float32)

    in_maps = []
    for b in range(B):
        ws = w_spatial[b].reshape(C, 8, NOFF)  # [i_glob, j_local, off]
        wsbd = np.zeros((NB, 128, NOFF, 128), np.float32)
        t = wsbd.reshape(NB, 16, 8, NOFF, 16, 8)
        wsv = ws.reshape(NB, 16, 8, 8, NOFF)  # [cb, g, ii, jj, off]
        for g in range(16):
            t[:, g, :, :, g, :] = wsv[:, g].transpose(0, 1, 3, 2)  # [cb, ii, off, jj]
        wp = w_pointwise[b][:, :, 0, 0].reshape(NB, 16, 8, 8)  # [cb, g, oo, ii]
        wptbd = np.zeros((NB, 128, 128), np.float32)
        t2 = wptbd.reshape(NB, 16, 8, 16, 8)
        for g in range(16):
            t2[:, g, :, g, :] = wp[:, g].transpose(0, 2, 1)  # [cb, ii, oo]
        in_maps.append(
            {
                "x": np.ascontiguousarray(x[b].reshape(C, HW)),
                "wsbd": wsbd,
                "wptbd": wptbd,
                "cwt": cwt,
                "biasp": np.ascontiguousarray(bias[b].reshape(NB, 128, 1)),
                "convbp": convbp,
            }
        )
    return in_maps


def kernel(x, w_spatial, w_pointwise, bias, conv_w, conv_b):
    global LAST_EXEC_NS
    if "nc" not in _CACHE:
        _CACHE["nc"] = _build()
    nc = _CACHE["nc"]
    in_maps = _prep_inputs(x, w_spatial, w_pointwise, bias, conv_w, conv_b)
    res = run_bass_kernel_spmd(nc, in_maps, core_ids=list(range(B)))
    LAST_EXEC_NS = res.exec_time_ns
    out = np.stack([r["out"] for r in res.results]).reshape(B, C, H, W)
    return out.astype(np.float32)
